# revision 10
# baseline (speedup 1.0000x reference)
"""MoE (token-choice top-2 router + grouped SwiGLU experts + shared expert)
on 8 Trainium2 NeuronCores.

Sharding: expert-parallel — core e owns expert e's routed tokens (host
dispatch, capacity-padded), plus a 1/8 data-parallel slice of the shared
expert. Host does the (cheap) routing control plane: gate matmul, top-2
selection, stable sort by expert, gather/scale, and the final scatter-add
combine. The device kernel does all the FLOPs: per-core SwiGLU
  h = silu(x @ w1.T) * (x @ w3.T);  out = h @ w2.T
in bf16 with fp32 PSUM accumulation (matching the reference's bf16
grouped-mm semantics), for both the routed tokens and the shared slice.

Self-contained: only needs numpy/ml_dtypes/concourse (the Bass stack).
"""

import math
import os

import numpy as np
import ml_dtypes

BF16 = ml_dtypes.bfloat16
NCORES = 8
TOP_K = 2
ROUTE_SCALE = 1.0

# filled by the last kernel() call (exec_time_ns etc. when tracing)
LAST = {}

_PROGRAM_CACHE = {}


def _install_profhook():
    """Best-effort shim for antenv.axon_hooks so trace=True can capture NTFF
    profiles in this container. Harmless no-op if anything is missing."""
    try:
        import sys
        import types

        if "antenv.axon_hooks" in sys.modules:
            return
        import trn_agent_boot.trn_boot as tb

        hook = tb._ntff_profile_via_ctypes("/opt/axon/libaxon_pjrt.so")
        m = types.ModuleType("antenv.axon_hooks")
        m._hook = hook
        m.set_axon_ntff_profile_hook = lambda h: setattr(m, "_hook", h)
        m.get_axon_ntff_profile_hook = lambda: m._hook
        import antenv

        sys.modules["antenv.axon_hooks"] = m
        antenv.axon_hooks = m

        import concourse.bass_utils as bu

        bu.upload_artifacts = lambda tmpdir: tmpdir
    except Exception:
        pass


def _free_div(n):
    """Largest f = n/k (k<=4) with f <= 512, preferring big f."""
    for k in (1, 2, 3, 4):
        if n % k == 0 and n // k <= 512:
            return n // k
    for f in (512, 384, 256, 128):
        if n % f == 0:
            return f
    raise ValueError(f"no free-dim divisor for {n}")


def _build_program(D, H, CAP, TS):
    import concourse.bacc as bacc
    import concourse.bass as bass
    import concourse.tile as tile
    from concourse import mybir
    from concourse.kernels.tile_matmul import (
        ShapeInfo,
        batched_producer_kxm,
        composable_matmul_tile_kernel,
        dma_from_dram_kxm,
        dma_from_dram_kxn,
        dma_to_dram_mxn,
    )
    from contextlib import ExitStack

    bf = mybir.dt.bfloat16
    f32 = mybir.dt.float32
    P = 128

    nc = bacc.Bacc(target_bir_lowering=False)

    xr = nc.dram_tensor("xr", [D, CAP], bf, kind="ExternalInput")
    w1t = nc.dram_tensor("w1t", [D, H], bf, kind="ExternalInput")
    w3t = nc.dram_tensor("w3t", [D, H], bf, kind="ExternalInput")
    w2t = nc.dram_tensor("w2t", [H, D], bf, kind="ExternalInput")
    xs = nc.dram_tensor("xs", [D, TS], bf, kind="ExternalInput")
    sw1t = nc.dram_tensor("sw1t", [D, H], bf, kind="ExternalInput")
    sw3t = nc.dram_tensor("sw3t", [D, H], bf, kind="ExternalInput")
    sw2t = nc.dram_tensor("sw2t", [H, D], bf, kind="ExternalInput")
    outr = nc.dram_tensor("outr", [CAP, D], f32, kind="ExternalOutput")
    outs = nc.dram_tensor("outs", [TS, D], f32, kind="ExternalOutput")

    with tile.TileContext(nc) as tc, ExitStack() as ctx:
        caches = ctx.enter_context(tc.tile_pool(name="caches", bufs=1))
        xcache = caches.tile([P, D // P, CAP], bf, tag="xcache")
        xscache = caches.tile([P, D // P, TS], bf, tag="xscache")
        h1cache = caches.tile([P, H // P, CAP], bf, tag="h1cache")
        gcache = caches.tile([P, H // P, CAP], bf, tag="gcache")
        h1scache = caches.tile([P, H // P, TS], bf, tag="h1scache")
        gscache = caches.tile([P, H // P, TS], bf, tag="gscache")

        def swiglu_h(label, w1ap, w3ap, xap, xc, h1c, gc, M_COLS):
            """h1c = silu(w1 @ x); gc = h1c * (w3 @ x). All [H, M_COLS]."""
            FREE = _free_div(M_COLS)
            kxm_pool = ctx.enter_context(tc.tile_pool(name=f"wp_{label}", bufs=8))
            kxn_pool = ctx.enter_context(tc.tile_pool(name=f"xp_{label}", bufs=1))
            p1, s1 = dma_from_dram_kxm(kxm_pool, w1ap[:])
            p3, s3 = dma_from_dram_kxm(kxm_pool, w3ap[:])
            kxm_producer, kxm_shape = batched_producer_kxm(
                [p1, p3], [s1, s3], batch_dim="m"
            )
            kxn_producer, kxn_shape = dma_from_dram_kxn(
                kxn_pool, xap[:], kxn_cache=xc
            )

            def producer(nc_, md):
                c = h1c if md.m_batch_idx == 0 else gc
                return c[
                    :,
                    bass.ts(md.m_tile_idx, md.m_subtiles),
                    bass.ts(md.n_tile_idx, md.n_tile),
                ]

            def reducer(nc_, psum, sbuf, md):
                if md.m_batch_idx == 0:
                    nc_.scalar.activation(
                        sbuf, psum, mybir.ActivationFunctionType.Silu
                    )
                else:
                    start = md.n_tile_idx * md.n_tile + md.n_subtile_idx * md.n_subtile
                    sz = md.n_subtile_slice_size
                    po = md.m_tile_idx * md.m_subtiles + md.m_subtile_idx
                    nc_.vector.tensor_mul(
                        out=sbuf,
                        in0=psum[:, :sz],
                        in1=h1c[:, po, start : start + sz],
                    )

            composable_matmul_tile_kernel(
                tc=tc,
                kxm_shape=kxm_shape,
                kxn_shape=kxn_shape,
                output_type=bf,
                kxm_producer=kxm_producer,
                kxn_producer=kxn_producer,
                mxn_subtile_producer=producer,
                mxn_subtile_reducer=reducer,
                mxn_consumer=lambda nc_, sbuf, md: None,
                MATMUL_FREE_DIM=FREE,
                MAX_TILE_SIZE=max(M_COLS, 128),
                MAX_K_TILE_SIZE=512,
            )

        def out_proj(label, gc, w2ap, out_ap, M_COLS, max_m):
            """out = (g.T @ w2.T) i.e. [M_COLS, D] = gT[H, M]^T @ w2T[H, D].
            gT lives in SBUF (gc) — producer is a pure slice, no DMA."""
            kxn_pool = ctx.enter_context(tc.tile_pool(name=f"w2p_{label}", bufs=13))

            def pm(nc_, md):
                return gc[
                    :,
                    bass.ts(md.k_tile_idx, md.k_subtiles),
                    bass.ts(md.m_tile_idx, md.m_tile),
                ]

            sm = ShapeInfo(pdims=((P, H // P),), fdims=(M_COLS,))
            pn, sn = dma_from_dram_kxn(kxn_pool, w2ap[:])
            consumer = dma_to_dram_mxn(out_ap[:])
            composable_matmul_tile_kernel(
                tc=tc,
                kxm_shape=sm,
                kxn_shape=sn,
                output_type=f32,
                kxm_producer=pm,
                kxn_producer=pn,
                mxn_consumer=consumer,
                MATMUL_FREE_DIM=512,
                MAX_TILE_SIZE=max_m,
                MAX_K_TILE_SIZE=512,
                temps_n_bufs=2,
            )

        swiglu_h("r", w1t, w3t, xr, xcache, h1cache, gcache, CAP)
        swiglu_h("s", sw1t, sw3t, xs, xscache, h1scache, gscache, TS)
        out_proj("r", gcache, w2t, outr, CAP, 512)
        out_proj("s", gscache, sw2t, outs, TS, 256)

    nc.compile()
    return nc


def _route(x, gate_w, expert_bias):
    """Host control plane mirroring the reference routing exactly."""
    BS, SLEN, D = x.shape
    T = BS * SLEN
    xt = np.ascontiguousarray(x.reshape(T, D), dtype=np.float32)
    logits = xt @ gate_w.astype(np.float32).T  # [T, E]
    scores = 1.0 / (1.0 + np.exp(-logits))
    biased = scores + np.asarray(expert_bias, np.float32)[None, :]
    sel = np.argsort(-biased, axis=1, kind="stable")[:, :TOP_K]  # [T, K]
    top_scores = np.take_along_axis(scores, sel, axis=1) * ROUTE_SCALE
    sel_flat = sel.reshape(-1)
    order = np.argsort(sel_flat, kind="stable")  # [T*K]
    counts = np.bincount(sel_flat, minlength=NCORES)
    tok_idx = order // TOP_K
    scores_sorted = top_scores.reshape(-1)[order].astype(np.float32)
    return xt, counts, tok_idx, scores_sorted


def kernel(x, gate_w, w1, w2, w3, sw1, sw2, sw3, expert_bias):
    from concourse.bass_utils import run_bass_kernel_spmd

    BS, SLEN, D = x.shape
    T = BS * SLEN
    H = w1.shape[1]
    TS = T // NCORES

    xt, counts, tok_idx, scores_sorted = _route(x, gate_w, expert_bias)
    off = np.concatenate([[0], np.cumsum(counts)]).astype(np.int64)
    CAP = max(128, int(math.ceil(counts.max() / 128) * 128))

    key = (D, H, CAP, TS)
    if key not in _PROGRAM_CACHE:
        _PROGRAM_CACHE[key] = _build_program(D, H, CAP, TS)
    nc = _PROGRAM_CACHE[key]

    # stage per-core inputs
    sw1t_h = np.ascontiguousarray(np.asarray(sw1, np.float32).T).astype(BF16)
    sw3t_h = np.ascontiguousarray(np.asarray(sw3, np.float32).T).astype(BF16)
    sw2t_h = np.ascontiguousarray(np.asarray(sw2, np.float32).T).astype(BF16)
    in_maps = []
    for e in range(NCORES):
        n_e = int(counts[e])
        idx = tok_idx[off[e] : off[e] + n_e]
        seg = xt[idx] * scores_sorted[off[e] : off[e] + n_e, None]  # [n_e, D] f32
        xrT = np.zeros((D, CAP), BF16)
        xrT[:, :n_e] = seg.T.astype(BF16)
        in_maps.append(
            {
                "xr": xrT,
                "w1t": np.ascontiguousarray(np.asarray(w1[e], np.float32).T).astype(BF16),
                "w3t": np.ascontiguousarray(np.asarray(w3[e], np.float32).T).astype(BF16),
                "w2t": np.ascontiguousarray(np.asarray(w2[e], np.float32).T).astype(BF16),
                "xs": np.ascontiguousarray(xt[e * TS : (e + 1) * TS].T).astype(BF16),
                "sw1t": sw1t_h,
                "sw3t": sw3t_h,
                "sw2t": sw2t_h,
            }
        )

    trace = bool(os.environ.get("KERNEL_TRACE"))
    if trace:
        _install_profhook()
    res = run_bass_kernel_spmd(
        nc, in_maps, list(range(NCORES)), trace=trace
    )
    LAST["exec_time_ns"] = res.exec_time_ns
    LAST["results"] = res

    # combine: shared slices + per-expert scatter-add
    out = np.empty((T, D), np.float32)
    for c in range(NCORES):
        out[c * TS : (c + 1) * TS] = res.results[c]["outs"]
    for e in range(NCORES):
        n_e = int(counts[e])
        if n_e:
            idx = tok_idx[off[e] : off[e] + n_e]
            out[idx] += res.results[e]["outr"][:n_e]
    return out.reshape(BS, SLEN, D)


# revision 32
# speedup vs baseline: 1.2230x; 1.2230x over previous
"""MoE (token-choice top-2 router + grouped SwiGLU experts + shared expert)
on 8 Trainium2 NeuronCores.

Sharding: expert-parallel — core e owns expert e's routed tokens (host
dispatch, capacity-padded), plus a 1/8 data-parallel slice of the shared
expert. Host does the (cheap) routing control plane: gate matmul, top-2
selection, stable sort by expert, gather/scale, and the final scatter-add
combine. The device kernel does all the FLOPs: per-core SwiGLU
  h = silu(x @ w1.T) * (x @ w3.T);  out = h @ w2.T
in bf16 with fp32 PSUM accumulation (matching the reference's bf16
grouped-mm semantics), for both the routed tokens and the shared slice.

Self-contained: only needs numpy/ml_dtypes/concourse (the Bass stack).
"""

import math
import os

import numpy as np
import ml_dtypes

BF16 = ml_dtypes.bfloat16
NCORES = 8
TOP_K = 2
ROUTE_SCALE = 1.0

# filled by the last kernel() call (exec_time_ns etc. when tracing)
LAST = {}

_PROGRAM_CACHE = {}


def _install_profhook():
    """Best-effort shim for antenv.axon_hooks so trace=True can capture NTFF
    profiles in this container. Harmless no-op if anything is missing."""
    try:
        import sys
        import types

        if "antenv.axon_hooks" in sys.modules:
            return
        import trn_agent_boot.trn_boot as tb

        hook = tb._ntff_profile_via_ctypes("/opt/axon/libaxon_pjrt.so")
        m = types.ModuleType("antenv.axon_hooks")
        m._hook = hook
        m.set_axon_ntff_profile_hook = lambda h: setattr(m, "_hook", h)
        m.get_axon_ntff_profile_hook = lambda: m._hook
        import antenv

        sys.modules["antenv.axon_hooks"] = m
        antenv.axon_hooks = m

        import concourse.bass_utils as bu

        bu.upload_artifacts = lambda tmpdir: tmpdir
    except Exception:
        pass


def _free_div(n):
    """Largest f = n/k (k<=4) with f <= 512, preferring big f."""
    for k in (1, 2, 3, 4):
        if n % k == 0 and n // k <= 512:
            return n // k
    for f in (512, 384, 256, 128):
        if n % f == 0:
            return f
    raise ValueError(f"no free-dim divisor for {n}")


def _build_program(D, H, CAP, TS):
    import concourse.bacc as bacc
    import concourse.bass as bass
    import concourse.tile as tile
    from concourse import mybir
    from concourse.kernels.tile_matmul import (
        ShapeInfo,
        batched_producer_kxm,
        composable_matmul_tile_kernel,
        dma_from_dram_kxm,
        dma_from_dram_kxn,
        dma_to_dram_mxn,
    )
    from contextlib import ExitStack

    bf = mybir.dt.bfloat16
    f32 = mybir.dt.float32
    P = 128

    nc = bacc.Bacc(target_bir_lowering=False)

    xr = nc.dram_tensor("xr", [D, CAP], bf, kind="ExternalInput")
    w1t = nc.dram_tensor("w1t", [D, H], bf, kind="ExternalInput")
    w3t = nc.dram_tensor("w3t", [D, H], bf, kind="ExternalInput")
    w2t = nc.dram_tensor("w2t", [H, D], bf, kind="ExternalInput")
    xs = nc.dram_tensor("xs", [D, TS], bf, kind="ExternalInput")
    sw1t = nc.dram_tensor("sw1t", [D, H], bf, kind="ExternalInput")
    sw3t = nc.dram_tensor("sw3t", [D, H], bf, kind="ExternalInput")
    sw2t = nc.dram_tensor("sw2t", [H, D], bf, kind="ExternalInput")
    outr = nc.dram_tensor("outr", [CAP, D], bf, kind="ExternalOutput")
    outs = nc.dram_tensor("outs", [TS, D], f32, kind="ExternalOutput")

    with tile.TileContext(nc) as tc, ExitStack() as ctx:
        caches = ctx.enter_context(tc.tile_pool(name="caches", bufs=1))
        xcache = caches.tile([P, D // P, CAP], bf, tag="xcache")
        xscache = caches.tile([P, D // P, TS], bf, tag="xscache")
        h1cache = caches.tile([P, H // P, CAP], bf, tag="h1cache")
        gcache = caches.tile([P, H // P, CAP], bf, tag="gcache")
        h1scache = caches.tile([P, H // P, TS], bf, tag="h1scache")
        gscache = caches.tile([P, H // P, TS], bf, tag="gscache")

        def swiglu_h(label, w1ap, w3ap, xap, xc, h1c, gc, M_COLS):
            """h1c = silu(w1 @ x); gc = h1c * (w3 @ x). All [H, M_COLS]."""
            FREE = _free_div(M_COLS)
            kxm_pool = ctx.enter_context(tc.tile_pool(name=f"wp_{label}", bufs=7))
            p1, s1 = dma_from_dram_kxm(kxm_pool, w1ap[:])
            p3, s3 = dma_from_dram_kxm(kxm_pool, w3ap[:])
            kxm_producer, kxm_shape = batched_producer_kxm(
                [p1, p3], [s1, s3], batch_dim="m"
            )

            x3 = xap[:].rearrange("(po pi) f -> pi po f", pi=P)

            def kxn_producer(nc_, md):
                # fill the SBUF cache with one DMA per k-subtile so the first
                # matmul only waits for its own 128-row slice
                cols = bass.ts(md.n_tile_idx, md.n_tile)
                for s in range(md.k_subtiles):
                    po = md.k_tile_idx * md.k_subtiles + s
                    nc_.sync.dma_start(
                        out=xc[:, po : po + 1, cols], in_=x3[:, po : po + 1, cols]
                    )
                return xc[:, bass.ts(md.k_tile_idx, md.k_subtiles), cols]

            kxn_shape = ShapeInfo(pdims=((P, D // P),), fdims=(M_COLS,))

            def producer(nc_, md):
                c = h1c if md.m_batch_idx == 0 else gc
                return c[
                    :,
                    bass.ts(md.m_tile_idx, md.m_subtiles),
                    bass.ts(md.n_tile_idx, md.n_tile),
                ]

            def reducer(nc_, psum, sbuf, md):
                if md.m_batch_idx == 0:
                    nc_.scalar.activation(
                        sbuf, psum, mybir.ActivationFunctionType.Silu
                    )
                else:
                    start = md.n_tile_idx * md.n_tile + md.n_subtile_idx * md.n_subtile
                    sz = md.n_subtile_slice_size
                    po = md.m_tile_idx * md.m_subtiles + md.m_subtile_idx
                    nc_.vector.tensor_mul(
                        out=sbuf,
                        in0=psum[:, :sz],
                        in1=h1c[:, po, start : start + sz],
                    )

            composable_matmul_tile_kernel(
                tc=tc,
                kxm_shape=kxm_shape,
                kxn_shape=kxn_shape,
                output_type=bf,
                kxm_producer=kxm_producer,
                kxn_producer=kxn_producer,
                mxn_subtile_producer=producer,
                mxn_subtile_reducer=reducer,
                mxn_consumer=lambda nc_, sbuf, md: None,
                MATMUL_FREE_DIM=FREE,
                MAX_TILE_SIZE=max(M_COLS, 128),
                MAX_K_TILE_SIZE=512,
                psum_n_bufs=2,
            )

        def out_proj(label, gc, w2ap, out_ap, out_dt, M_COLS, max_m, big_kxn, psum_bufs=2):
            """out = (g.T @ w2.T) i.e. [M_COLS, D] = gT[H, M]^T @ w2T[H, D].
            gT lives in SBUF (gc) — kxm producer is a pure slice, no DMA."""

            def pm(nc_, md):
                return gc[
                    :,
                    bass.ts(md.k_tile_idx, md.k_subtiles),
                    bass.ts(md.m_tile_idx, md.m_tile),
                ]

            sm = ShapeInfo(pdims=((P, H // P),), fdims=(M_COLS,))

            # W2 strips in k-groups of <=4, issued from GpSimd (SP's DMA
            # descriptor-issue rate saturates in this phase otherwise)
            KT = H // P  # k-tiles (K_TILE=128)
            GRP = 4
            w2pool = ctx.enter_context(
                tc.tile_pool(name=f"w2p_{label}", bufs=2)
            )
            w2_3d = w2ap[:].rearrange("(po pi) f -> pi po f", pi=P)
            state = {"n": None, "grp": {}}

            def pn(nc_, md):
                if state["n"] != md.n_tile_idx:
                    state["n"] = md.n_tile_idx
                    state["grp"] = {}
                    cols = bass.ts(md.n_tile_idx, md.n_tile)
                    for g0 in range(0, KT, GRP):
                        g1 = min(g0 + GRP, KT)
                        t = w2pool.tile(
                            [P, g1 - g0, md.n_tile], bf, tag=f"w2g_{label}_{g0}"
                        )
                        if label == "r":
                            nc_.gpsimd.dma_start(out=t[:], in_=w2_3d[:, g0:g1, cols])
                        else:
                            nc_.scalar.dma_start(out=t[:], in_=w2_3d[:, g0:g1, cols])
                        for k in range(g0, g1):
                            state["grp"][k] = t[:, k - g0 : k - g0 + 1, :]
                return state["grp"][md.k_tile_idx]

            sn = ShapeInfo(pdims=((P, H // P),), fdims=(D,))

            out3 = out_ap[:].rearrange("(po pi) f -> pi po f", pi=P)

            def consumer(nc_, mxn_tile, md):
                n_sz = min(md.n_tile, D - md.n_tile_idx * md.n_tile)
                eng = nc_.scalar if label == "r" else nc_.sync
                eng.dma_start(
                    out=out3[
                        :,
                        bass.ts(md.m_tile_idx, md.m_subtiles),
                        bass.ds(md.n_tile_idx * md.n_tile, n_sz),
                    ],
                    in_=mxn_tile[:, :, :n_sz],
                )
            composable_matmul_tile_kernel(
                tc=tc,
                kxm_shape=sm,
                kxn_shape=sn,
                output_type=out_dt,
                kxm_producer=pm,
                kxn_producer=pn,
                mxn_consumer=consumer,
                MATMUL_FREE_DIM=512,
                MAX_TILE_SIZE=max_m,
                MAX_K_TILE_SIZE=512,
                temps_n_bufs=2,
                psum_n_bufs=psum_bufs,
            )

        swiglu_h("r", w1t, w3t, xr, xcache, h1cache, gcache, CAP)
        swiglu_h("s", sw1t, sw3t, xs, xscache, h1scache, gscache, TS)
        out_proj("r", gcache, w2t, outr, bf, CAP, 512, big_kxn=True, psum_bufs=2)
        out_proj("s", gscache, sw2t, outs, f32, TS, 256, big_kxn=True, psum_bufs=2)

    nc.compile()
    return nc


def _route(x, gate_w, expert_bias):
    """Host control plane mirroring the reference routing exactly."""
    BS, SLEN, D = x.shape
    T = BS * SLEN
    xt = np.ascontiguousarray(x.reshape(T, D), dtype=np.float32)
    logits = xt @ gate_w.astype(np.float32).T  # [T, E]
    scores = 1.0 / (1.0 + np.exp(-logits))
    biased = scores + np.asarray(expert_bias, np.float32)[None, :]
    sel = np.argsort(-biased, axis=1, kind="stable")[:, :TOP_K]  # [T, K]
    top_scores = np.take_along_axis(scores, sel, axis=1) * ROUTE_SCALE
    sel_flat = sel.reshape(-1)
    order = np.argsort(sel_flat, kind="stable")  # [T*K]
    counts = np.bincount(sel_flat, minlength=NCORES)
    tok_idx = order // TOP_K
    scores_sorted = top_scores.reshape(-1)[order].astype(np.float32)
    return xt, counts, tok_idx, scores_sorted


def kernel(x, gate_w, w1, w2, w3, sw1, sw2, sw3, expert_bias):
    from concourse.bass_utils import run_bass_kernel_spmd

    x = np.asarray(x, np.float32)
    gate_w = np.asarray(gate_w, np.float32)
    w1 = np.asarray(w1, np.float32)
    w2 = np.asarray(w2, np.float32)
    w3 = np.asarray(w3, np.float32)
    sw1 = np.asarray(sw1, np.float32)
    sw2 = np.asarray(sw2, np.float32)
    sw3 = np.asarray(sw3, np.float32)
    expert_bias = np.asarray(expert_bias, np.float32)
    BS, SLEN, D = x.shape
    T = BS * SLEN
    H = w1.shape[1]
    TS = T // NCORES

    xt, counts, tok_idx, scores_sorted = _route(x, gate_w, expert_bias)
    off = np.concatenate([[0], np.cumsum(counts)]).astype(np.int64)
    CAP = max(128, int(math.ceil(counts.max() / 128) * 128))

    key = (D, H, CAP, TS)
    if key not in _PROGRAM_CACHE:
        _PROGRAM_CACHE[key] = _build_program(D, H, CAP, TS)
    nc = _PROGRAM_CACHE[key]

    # stage per-core inputs
    sw1t_h = np.ascontiguousarray(np.asarray(sw1, np.float32).T).astype(BF16)
    sw3t_h = np.ascontiguousarray(np.asarray(sw3, np.float32).T).astype(BF16)
    sw2t_h = np.ascontiguousarray(np.asarray(sw2, np.float32).T).astype(BF16)
    in_maps = []
    for e in range(NCORES):
        n_e = int(counts[e])
        idx = tok_idx[off[e] : off[e] + n_e]
        seg = xt[idx] * scores_sorted[off[e] : off[e] + n_e, None]  # [n_e, D] f32
        xrT = np.zeros((D, CAP), BF16)
        xrT[:, :n_e] = seg.T.astype(BF16)
        in_maps.append(
            {
                "xr": xrT,
                "w1t": np.ascontiguousarray(np.asarray(w1[e], np.float32).T).astype(BF16),
                "w3t": np.ascontiguousarray(np.asarray(w3[e], np.float32).T).astype(BF16),
                "w2t": np.ascontiguousarray(np.asarray(w2[e], np.float32).T).astype(BF16),
                "xs": np.ascontiguousarray(xt[e * TS : (e + 1) * TS].T).astype(BF16),
                "sw1t": sw1t_h,
                "sw3t": sw3t_h,
                "sw2t": sw2t_h,
            }
        )

    trace = os.environ.get("KERNEL_TRACE", "") not in ("", "0")
    if trace:
        _install_profhook()
    res = run_bass_kernel_spmd(
        nc, in_maps, list(range(NCORES)), trace=trace
    )
    LAST["exec_time_ns"] = res.exec_time_ns
    LAST["results"] = res

    # combine: shared slices + per-expert scatter-add
    out = np.empty((T, D), np.float32)
    for c in range(NCORES):
        out[c * TS : (c + 1) * TS] = res.results[c]["outs"]
    for e in range(NCORES):
        n_e = int(counts[e])
        if n_e:
            idx = tok_idx[off[e] : off[e] + n_e]
            out[idx] += res.results[e]["outr"][:n_e]
    return out.reshape(BS, SLEN, D)


# revision 36
# speedup vs baseline: 1.2445x; 1.0176x over previous
"""MoE (token-choice top-2 router + grouped SwiGLU experts + shared expert)
on 8 Trainium2 NeuronCores.

Sharding: expert-parallel — core e owns expert e's routed tokens (host
dispatch, capacity-padded), plus a 1/8 data-parallel slice of the shared
expert. Host does the (cheap) routing control plane: gate matmul, top-2
selection, stable sort by expert, gather/scale, and the final scatter-add
combine. The device kernel does all the FLOPs: per-core SwiGLU
  h = silu(x @ w1.T) * (x @ w3.T);  out = h @ w2.T
in bf16 with fp32 PSUM accumulation (matching the reference's bf16
grouped-mm semantics), for both the routed tokens and the shared slice.

Self-contained: only needs numpy/ml_dtypes/concourse (the Bass stack).
"""

import math
import os

import numpy as np
import ml_dtypes

BF16 = ml_dtypes.bfloat16
NCORES = 8
TOP_K = 2
ROUTE_SCALE = 1.0

# filled by the last kernel() call (exec_time_ns etc. when tracing)
LAST = {}

_PROGRAM_CACHE = {}


def _install_profhook():
    """Best-effort shim for antenv.axon_hooks so trace=True can capture NTFF
    profiles in this container. Harmless no-op if anything is missing."""
    try:
        import sys
        import types

        if "antenv.axon_hooks" in sys.modules:
            return
        import trn_agent_boot.trn_boot as tb

        hook = tb._ntff_profile_via_ctypes("/opt/axon/libaxon_pjrt.so")
        m = types.ModuleType("antenv.axon_hooks")
        m._hook = hook
        m.set_axon_ntff_profile_hook = lambda h: setattr(m, "_hook", h)
        m.get_axon_ntff_profile_hook = lambda: m._hook
        import antenv

        sys.modules["antenv.axon_hooks"] = m
        antenv.axon_hooks = m

        import concourse.bass_utils as bu

        bu.upload_artifacts = lambda tmpdir: tmpdir
    except Exception:
        pass


def _free_div(n):
    """Largest f = n/k (k<=4) with f <= 512, preferring big f."""
    for k in (1, 2, 3, 4):
        if n % k == 0 and n // k <= 512:
            return n // k
    for f in (512, 384, 256, 128):
        if n % f == 0:
            return f
    raise ValueError(f"no free-dim divisor for {n}")


def _pick_ntok(nmax, cap):
    """Smallest n in [nmax, cap] whose free-dim divides nicely (PSUM <=512)."""
    for n in range(nmax, cap + 1):
        try:
            _free_div(n)
            return n
        except ValueError:
            continue
    return cap


def _build_program(D, H, CAP, NTOK, TS):
    import concourse.bacc as bacc
    import concourse.bass as bass
    import concourse.tile as tile
    from concourse import mybir
    from concourse.kernels.tile_matmul import (
        ShapeInfo,
        batched_producer_kxm,
        composable_matmul_tile_kernel,
        dma_from_dram_kxm,
        dma_from_dram_kxn,
        dma_to_dram_mxn,
    )
    from contextlib import ExitStack

    bf = mybir.dt.bfloat16
    f32 = mybir.dt.float32
    P = 128

    nc = bacc.Bacc(target_bir_lowering=False)

    xr = nc.dram_tensor("xr", [D, NTOK], bf, kind="ExternalInput")
    w1t = nc.dram_tensor("w1t", [D, H], bf, kind="ExternalInput")
    w3t = nc.dram_tensor("w3t", [D, H], bf, kind="ExternalInput")
    w2t = nc.dram_tensor("w2t", [H, D], bf, kind="ExternalInput")
    xs = nc.dram_tensor("xs", [D, TS], bf, kind="ExternalInput")
    sw1t = nc.dram_tensor("sw1t", [D, H], bf, kind="ExternalInput")
    sw3t = nc.dram_tensor("sw3t", [D, H], bf, kind="ExternalInput")
    sw2t = nc.dram_tensor("sw2t", [H, D], bf, kind="ExternalInput")
    outr = nc.dram_tensor("outr", [CAP, D], bf, kind="ExternalOutput")
    outs = nc.dram_tensor("outs", [TS, D], f32, kind="ExternalOutput")

    with tile.TileContext(nc) as tc, ExitStack() as ctx:
        caches = ctx.enter_context(tc.tile_pool(name="caches", bufs=1))
        xcache = caches.tile([P, D // P, NTOK], bf, tag="xcache")
        xscache = caches.tile([P, D // P, TS], bf, tag="xscache")
        h1cache = caches.tile([P, H // P, NTOK], bf, tag="h1cache")
        gcache = caches.tile([P, H // P, CAP], bf, tag="gcache")
        h1scache = caches.tile([P, H // P, TS], bf, tag="h1scache")
        gscache = caches.tile([P, H // P, TS], bf, tag="gscache")

        def swiglu_h(label, w1ap, w3ap, xap, xc, h1c, gc, M_COLS):
            """h1c = silu(w1 @ x); gc = h1c * (w3 @ x). All [H, M_COLS]."""
            FREE = _free_div(M_COLS)
            kxm_pool = ctx.enter_context(tc.tile_pool(name=f"wp_{label}", bufs=7))
            p1, s1 = dma_from_dram_kxm(kxm_pool, w1ap[:])
            p3, s3 = dma_from_dram_kxm(kxm_pool, w3ap[:])
            kxm_producer, kxm_shape = batched_producer_kxm(
                [p1, p3], [s1, s3], batch_dim="m"
            )

            x3 = xap[:].rearrange("(po pi) f -> pi po f", pi=P)

            def kxn_producer(nc_, md):
                # fill the SBUF cache with one DMA per k-subtile so the first
                # matmul only waits for its own 128-row slice
                cols = bass.ts(md.n_tile_idx, md.n_tile)
                for s in range(md.k_subtiles):
                    po = md.k_tile_idx * md.k_subtiles + s
                    nc_.sync.dma_start(
                        out=xc[:, po : po + 1, cols], in_=x3[:, po : po + 1, cols]
                    )
                return xc[:, bass.ts(md.k_tile_idx, md.k_subtiles), cols]

            kxn_shape = ShapeInfo(pdims=((P, D // P),), fdims=(M_COLS,))

            def producer(nc_, md):
                c = h1c if md.m_batch_idx == 0 else gc
                return c[
                    :,
                    bass.ts(md.m_tile_idx, md.m_subtiles),
                    bass.ts(md.n_tile_idx, md.n_tile),
                ]

            def reducer(nc_, psum, sbuf, md):
                if md.m_batch_idx == 0:
                    nc_.scalar.activation(
                        sbuf, psum, mybir.ActivationFunctionType.Silu
                    )
                else:
                    start = md.n_tile_idx * md.n_tile + md.n_subtile_idx * md.n_subtile
                    sz = md.n_subtile_slice_size
                    po = md.m_tile_idx * md.m_subtiles + md.m_subtile_idx
                    nc_.vector.tensor_mul(
                        out=sbuf,
                        in0=psum[:, :sz],
                        in1=h1c[:, po, start : start + sz],
                    )

            composable_matmul_tile_kernel(
                tc=tc,
                kxm_shape=kxm_shape,
                kxn_shape=kxn_shape,
                output_type=bf,
                kxm_producer=kxm_producer,
                kxn_producer=kxn_producer,
                mxn_subtile_producer=producer,
                mxn_subtile_reducer=reducer,
                mxn_consumer=lambda nc_, sbuf, md: None,
                MATMUL_FREE_DIM=FREE,
                MAX_TILE_SIZE=max(M_COLS, 128),
                MAX_K_TILE_SIZE=512,
                psum_n_bufs=2,
            )

        def out_proj(label, gc, w2ap, out_ap, out_dt, M_COLS, max_m, big_kxn, psum_bufs=2):
            """out = (g.T @ w2.T) i.e. [M_COLS, D] = gT[H, M]^T @ w2T[H, D].
            gT lives in SBUF (gc) — kxm producer is a pure slice, no DMA."""

            def pm(nc_, md):
                return gc[
                    :,
                    bass.ts(md.k_tile_idx, md.k_subtiles),
                    bass.ts(md.m_tile_idx, md.m_tile),
                ]

            sm = ShapeInfo(pdims=((P, H // P),), fdims=(M_COLS,))

            # W2 strips in k-groups of <=4, issued from GpSimd (SP's DMA
            # descriptor-issue rate saturates in this phase otherwise)
            KT = H // P  # k-tiles (K_TILE=128)
            GRP = 4
            w2pool = ctx.enter_context(
                tc.tile_pool(name=f"w2p_{label}", bufs=2)
            )
            w2_3d = w2ap[:].rearrange("(po pi) f -> pi po f", pi=P)
            state = {"n": None, "grp": {}}

            def pn(nc_, md):
                if state["n"] != md.n_tile_idx:
                    state["n"] = md.n_tile_idx
                    state["grp"] = {}
                    cols = bass.ts(md.n_tile_idx, md.n_tile)
                    for g0 in range(0, KT, GRP):
                        g1 = min(g0 + GRP, KT)
                        t = w2pool.tile(
                            [P, g1 - g0, md.n_tile], bf, tag=f"w2g_{label}_{g0}"
                        )
                        if label == "r":
                            nc_.gpsimd.dma_start(out=t[:], in_=w2_3d[:, g0:g1, cols])
                        else:
                            nc_.scalar.dma_start(out=t[:], in_=w2_3d[:, g0:g1, cols])
                        for k in range(g0, g1):
                            state["grp"][k] = t[:, k - g0 : k - g0 + 1, :]
                return state["grp"][md.k_tile_idx]

            sn = ShapeInfo(pdims=((P, H // P),), fdims=(D,))

            out3 = out_ap[:].rearrange("(po pi) f -> pi po f", pi=P)

            def consumer(nc_, mxn_tile, md):
                n_sz = min(md.n_tile, D - md.n_tile_idx * md.n_tile)
                eng = nc_.scalar if label == "r" else nc_.sync
                eng.dma_start(
                    out=out3[
                        :,
                        bass.ts(md.m_tile_idx, md.m_subtiles),
                        bass.ds(md.n_tile_idx * md.n_tile, n_sz),
                    ],
                    in_=mxn_tile[:, :, :n_sz],
                )
            composable_matmul_tile_kernel(
                tc=tc,
                kxm_shape=sm,
                kxn_shape=sn,
                output_type=out_dt,
                kxm_producer=pm,
                kxn_producer=pn,
                mxn_consumer=consumer,
                MATMUL_FREE_DIM=512,
                MAX_TILE_SIZE=max_m,
                MAX_K_TILE_SIZE=512,
                temps_n_bufs=2,
                psum_n_bufs=psum_bufs,
            )

        swiglu_h("r", w1t, w3t, xr, xcache, h1cache, gcache, NTOK)
        swiglu_h("s", sw1t, sw3t, xs, xscache, h1scache, gscache, TS)
        out_proj("r", gcache, w2t, outr, bf, CAP, 512, big_kxn=True, psum_bufs=2)
        out_proj("s", gscache, sw2t, outs, f32, TS, 256, big_kxn=True, psum_bufs=2)

    nc.compile()
    return nc


def _route(x, gate_w, expert_bias):
    """Host control plane mirroring the reference routing exactly."""
    BS, SLEN, D = x.shape
    T = BS * SLEN
    xt = np.ascontiguousarray(x.reshape(T, D), dtype=np.float32)
    logits = xt @ gate_w.astype(np.float32).T  # [T, E]
    scores = 1.0 / (1.0 + np.exp(-logits))
    biased = scores + np.asarray(expert_bias, np.float32)[None, :]
    sel = np.argsort(-biased, axis=1, kind="stable")[:, :TOP_K]  # [T, K]
    top_scores = np.take_along_axis(scores, sel, axis=1) * ROUTE_SCALE
    sel_flat = sel.reshape(-1)
    order = np.argsort(sel_flat, kind="stable")  # [T*K]
    counts = np.bincount(sel_flat, minlength=NCORES)
    tok_idx = order // TOP_K
    scores_sorted = top_scores.reshape(-1)[order].astype(np.float32)
    return xt, counts, tok_idx, scores_sorted


def kernel(x, gate_w, w1, w2, w3, sw1, sw2, sw3, expert_bias):
    from concourse.bass_utils import run_bass_kernel_spmd

    x = np.asarray(x, np.float32)
    gate_w = np.asarray(gate_w, np.float32)
    w1 = np.asarray(w1, np.float32)
    w2 = np.asarray(w2, np.float32)
    w3 = np.asarray(w3, np.float32)
    sw1 = np.asarray(sw1, np.float32)
    sw2 = np.asarray(sw2, np.float32)
    sw3 = np.asarray(sw3, np.float32)
    expert_bias = np.asarray(expert_bias, np.float32)
    BS, SLEN, D = x.shape
    T = BS * SLEN
    H = w1.shape[1]
    TS = T // NCORES

    xt, counts, tok_idx, scores_sorted = _route(x, gate_w, expert_bias)
    off = np.concatenate([[0], np.cumsum(counts)]).astype(np.int64)
    CAP = max(128, int(math.ceil(counts.max() / 128) * 128))
    NTOK = _pick_ntok(max(128, int(counts.max())), CAP)

    key = (D, H, CAP, NTOK, TS)
    if key not in _PROGRAM_CACHE:
        _PROGRAM_CACHE[key] = _build_program(D, H, CAP, NTOK, TS)
    nc = _PROGRAM_CACHE[key]

    # stage per-core inputs
    sw1t_h = np.ascontiguousarray(np.asarray(sw1, np.float32).T).astype(BF16)
    sw3t_h = np.ascontiguousarray(np.asarray(sw3, np.float32).T).astype(BF16)
    sw2t_h = np.ascontiguousarray(np.asarray(sw2, np.float32).T).astype(BF16)
    in_maps = []
    for e in range(NCORES):
        n_e = int(counts[e])
        idx = tok_idx[off[e] : off[e] + n_e]
        seg = xt[idx] * scores_sorted[off[e] : off[e] + n_e, None]  # [n_e, D] f32
        xrT = np.zeros((D, NTOK), BF16)
        xrT[:, :n_e] = seg.T.astype(BF16)
        in_maps.append(
            {
                "xr": xrT,
                "w1t": np.ascontiguousarray(np.asarray(w1[e], np.float32).T).astype(BF16),
                "w3t": np.ascontiguousarray(np.asarray(w3[e], np.float32).T).astype(BF16),
                "w2t": np.ascontiguousarray(np.asarray(w2[e], np.float32).T).astype(BF16),
                "xs": np.ascontiguousarray(xt[e * TS : (e + 1) * TS].T).astype(BF16),
                "sw1t": sw1t_h,
                "sw3t": sw3t_h,
                "sw2t": sw2t_h,
            }
        )

    trace = os.environ.get("KERNEL_TRACE", "") not in ("", "0")
    if trace:
        _install_profhook()
    res = run_bass_kernel_spmd(
        nc, in_maps, list(range(NCORES)), trace=trace
    )
    LAST["exec_time_ns"] = res.exec_time_ns
    LAST["results"] = res

    # combine: shared slices + per-expert scatter-add
    out = np.empty((T, D), np.float32)
    for c in range(NCORES):
        out[c * TS : (c + 1) * TS] = res.results[c]["outs"]
    for e in range(NCORES):
        n_e = int(counts[e])
        if n_e:
            idx = tok_idx[off[e] : off[e] + n_e]
            out[idx] += res.results[e]["outr"][:n_e]
    return out.reshape(BS, SLEN, D)


# revision 42
# speedup vs baseline: 1.2557x; 1.0090x over previous
"""MoE (token-choice top-2 router + grouped SwiGLU experts + shared expert)
on 8 Trainium2 NeuronCores.

Sharding: expert-parallel — core e owns expert e's routed tokens (host
dispatch, capacity-padded), plus a 1/8 data-parallel slice of the shared
expert. Host does the (cheap) routing control plane: gate matmul, top-2
selection, stable sort by expert, gather/scale, and the final scatter-add
combine. The device kernel does all the FLOPs: per-core SwiGLU
  h = silu(x @ w1.T) * (x @ w3.T);  out = h @ w2.T
in bf16 with fp32 PSUM accumulation (matching the reference's bf16
grouped-mm semantics), for both the routed tokens and the shared slice.

Self-contained: only needs numpy/ml_dtypes/concourse (the Bass stack).
"""

import math
import os

import numpy as np
import ml_dtypes

BF16 = ml_dtypes.bfloat16
NCORES = 8
TOP_K = 2
ROUTE_SCALE = 1.0

# filled by the last kernel() call (exec_time_ns etc. when tracing)
LAST = {}

_PROGRAM_CACHE = {}


def _install_profhook():
    """Best-effort shim for antenv.axon_hooks so trace=True can capture NTFF
    profiles in this container. Harmless no-op if anything is missing."""
    try:
        import sys
        import types

        if "antenv.axon_hooks" in sys.modules:
            return
        import trn_agent_boot.trn_boot as tb

        hook = tb._ntff_profile_via_ctypes("/opt/axon/libaxon_pjrt.so")
        m = types.ModuleType("antenv.axon_hooks")
        m._hook = hook
        m.set_axon_ntff_profile_hook = lambda h: setattr(m, "_hook", h)
        m.get_axon_ntff_profile_hook = lambda: m._hook
        import antenv

        sys.modules["antenv.axon_hooks"] = m
        antenv.axon_hooks = m

        import concourse.bass_utils as bu

        bu.upload_artifacts = lambda tmpdir: tmpdir
    except Exception:
        pass


def _free_div(n):
    """Largest f = n/k (k<=4) with f <= 512, preferring big f."""
    for k in (1, 2, 3, 4):
        if n % k == 0 and n // k <= 512:
            return n // k
    for f in (512, 384, 256, 128):
        if n % f == 0:
            return f
    raise ValueError(f"no free-dim divisor for {n}")


def _pick_ntok(nmax, cap):
    """Smallest n in [nmax, cap] whose free-dim divides nicely (PSUM <=512)."""
    for n in range(nmax, cap + 1):
        try:
            _free_div(n)
            return n
        except ValueError:
            continue
    return cap


def _build_program(D, H, CAP, NTOK, TS):
    import concourse.bacc as bacc
    import concourse.bass as bass
    import concourse.tile as tile
    from concourse import mybir
    from concourse.kernels.tile_matmul import (
        ShapeInfo,
        batched_producer_kxm,
        composable_matmul_tile_kernel,
        dma_from_dram_kxm,
        dma_from_dram_kxn,
        dma_to_dram_mxn,
    )
    from contextlib import ExitStack

    bf = mybir.dt.bfloat16
    f32 = mybir.dt.float32
    P = 128

    nc = bacc.Bacc(target_bir_lowering=False)

    xr = nc.dram_tensor("xr", [D, NTOK], bf, kind="ExternalInput")
    w1t = nc.dram_tensor("w1t", [D, H], bf, kind="ExternalInput")
    w3t = nc.dram_tensor("w3t", [D, H], bf, kind="ExternalInput")
    w2t = nc.dram_tensor("w2t", [H, D], bf, kind="ExternalInput")
    xs = nc.dram_tensor("xs", [D, TS], bf, kind="ExternalInput")
    sw1t = nc.dram_tensor("sw1t", [D, H], bf, kind="ExternalInput")
    sw3t = nc.dram_tensor("sw3t", [D, H], bf, kind="ExternalInput")
    sw2t = nc.dram_tensor("sw2t", [H, D], bf, kind="ExternalInput")
    outr = nc.dram_tensor("outr", [CAP, D], bf, kind="ExternalOutput")
    outs = nc.dram_tensor("outs", [TS, D], f32, kind="ExternalOutput")

    with tile.TileContext(nc) as tc, ExitStack() as ctx:
        caches = ctx.enter_context(tc.tile_pool(name="caches", bufs=1))
        xcache = caches.tile([P, D // P, NTOK], bf, tag="xcache")
        xscache = caches.tile([P, D // P, TS], bf, tag="xscache")
        h1cache = caches.tile([P, H // P, NTOK], bf, tag="h1cache")
        gcache = caches.tile([P, H // P, CAP], bf, tag="gcache")
        h1scache = caches.tile([P, H // P, TS], bf, tag="h1scache")
        gscache = caches.tile([P, H // P, TS], bf, tag="gscache")

        def swiglu_h(label, w1ap, w3ap, xap, xc, h1c, gc, M_COLS):
            """h1c = silu(w1 @ x); gc = h1c * (w3 @ x). All [H, M_COLS]."""
            FREE = _free_div(M_COLS)
            kxm_pool = ctx.enter_context(tc.tile_pool(name=f"wp_{label}", bufs=7))
            p1, s1 = dma_from_dram_kxm(kxm_pool, w1ap[:])
            p3, s3 = dma_from_dram_kxm(kxm_pool, w3ap[:])
            kxm_producer, kxm_shape = batched_producer_kxm(
                [p1, p3], [s1, s3], batch_dim="m"
            )

            x3 = xap[:].rearrange("(po pi) f -> pi po f", pi=P)

            def kxn_producer(nc_, md):
                # fill the SBUF cache with one DMA per k-subtile so the first
                # matmul only waits for its own 128-row slice
                cols = bass.ts(md.n_tile_idx, md.n_tile)
                for s in range(md.k_subtiles):
                    po = md.k_tile_idx * md.k_subtiles + s
                    nc_.sync.dma_start(
                        out=xc[:, po : po + 1, cols], in_=x3[:, po : po + 1, cols]
                    )
                return xc[:, bass.ts(md.k_tile_idx, md.k_subtiles), cols]

            kxn_shape = ShapeInfo(pdims=((P, D // P),), fdims=(M_COLS,))

            def producer(nc_, md):
                c = h1c if md.m_batch_idx == 0 else gc
                return c[
                    :,
                    bass.ts(md.m_tile_idx, md.m_subtiles),
                    bass.ts(md.n_tile_idx, md.n_tile),
                ]

            def reducer(nc_, psum, sbuf, md):
                if md.m_batch_idx == 0:
                    nc_.scalar.activation(
                        sbuf, psum, mybir.ActivationFunctionType.Silu
                    )
                else:
                    start = md.n_tile_idx * md.n_tile + md.n_subtile_idx * md.n_subtile
                    sz = md.n_subtile_slice_size
                    po = md.m_tile_idx * md.m_subtiles + md.m_subtile_idx
                    nc_.vector.tensor_mul(
                        out=sbuf,
                        in0=psum[:, :sz],
                        in1=h1c[:, po, start : start + sz],
                    )

            composable_matmul_tile_kernel(
                tc=tc,
                kxm_shape=kxm_shape,
                kxn_shape=kxn_shape,
                output_type=bf,
                kxm_producer=kxm_producer,
                kxn_producer=kxn_producer,
                mxn_subtile_producer=producer,
                mxn_subtile_reducer=reducer,
                mxn_consumer=lambda nc_, sbuf, md: None,
                MATMUL_FREE_DIM=FREE,
                MAX_TILE_SIZE=max(M_COLS, 128),
                MAX_K_TILE_SIZE=512,
                psum_n_bufs=2,
            )

        def out_proj(label, gc, w2ap, out_ap, out_dt, M_COLS, max_m, big_kxn, psum_bufs=2):
            """out = (g.T @ w2.T) i.e. [M_COLS, D] = gT[H, M]^T @ w2T[H, D].
            gT lives in SBUF (gc) — kxm producer is a pure slice, no DMA."""

            def pm(nc_, md):
                return gc[
                    :,
                    bass.ts(md.k_tile_idx, md.k_subtiles),
                    bass.ts(md.m_tile_idx, md.m_tile),
                ]

            sm = ShapeInfo(pdims=((P, H // P),), fdims=(M_COLS,))

            # W2 strips in k-groups of <=4, issued from GpSimd (SP's DMA
            # descriptor-issue rate saturates in this phase otherwise)
            KT = H // P  # k-tiles (K_TILE=128)
            GRP = 4
            w2pool = ctx.enter_context(
                tc.tile_pool(name=f"w2p_{label}", bufs=2)
            )
            w2_3d = w2ap[:].rearrange("(po pi) f -> pi po f", pi=P)
            state = {"n": None, "grp": {}}

            def pn(nc_, md):
                if state["n"] != md.n_tile_idx:
                    state["n"] = md.n_tile_idx
                    state["grp"] = {}
                    cols = bass.ts(md.n_tile_idx, md.n_tile)
                    for g0 in range(0, KT, GRP):
                        g1 = min(g0 + GRP, KT)
                        t = w2pool.tile(
                            [P, g1 - g0, md.n_tile], bf, tag=f"w2g_{label}_{g0}"
                        )
                        if label == "r":
                            nc_.gpsimd.dma_start(out=t[:], in_=w2_3d[:, g0:g1, cols])
                        else:
                            nc_.scalar.dma_start(out=t[:], in_=w2_3d[:, g0:g1, cols])
                        for k in range(g0, g1):
                            state["grp"][k] = t[:, k - g0 : k - g0 + 1, :]
                return state["grp"][md.k_tile_idx]

            sn = ShapeInfo(pdims=((P, H // P),), fdims=(D,))

            out3 = out_ap[:].rearrange("(po pi) f -> pi po f", pi=P)

            def consumer(nc_, mxn_tile, md):
                n_sz = min(md.n_tile, D - md.n_tile_idx * md.n_tile)
                eng = nc_.scalar if label == "r" else nc_.sync
                eng.dma_start(
                    out=out3[
                        :,
                        bass.ts(md.m_tile_idx, md.m_subtiles),
                        bass.ds(md.n_tile_idx * md.n_tile, n_sz),
                    ],
                    in_=mxn_tile[:, :, :n_sz],
                )
            composable_matmul_tile_kernel(
                tc=tc,
                kxm_shape=sm,
                kxn_shape=sn,
                output_type=out_dt,
                kxm_producer=pm,
                kxn_producer=pn,
                mxn_consumer=consumer,
                MATMUL_FREE_DIM=512,
                MAX_TILE_SIZE=max_m,
                MAX_K_TILE_SIZE=512,
                temps_n_bufs=2,
                psum_n_bufs=psum_bufs,
            )

        swiglu_h("r", w1t, w3t, xr, xcache, h1cache, gcache, NTOK)
        swiglu_h("s", sw1t, sw3t, xs, xscache, h1scache, gscache, TS)
        out_proj("r", gcache, w2t, outr, bf, CAP, 512, big_kxn=True, psum_bufs=2)
        out_proj("s", gscache, sw2t, outs, f32, TS, 256, big_kxn=True, psum_bufs=2)

    nc.compile()
    return nc


def _route(x, gate_w, expert_bias):
    """Host control plane mirroring the reference routing exactly."""
    BS, SLEN, D = x.shape
    T = BS * SLEN
    xt = np.ascontiguousarray(x.reshape(T, D), dtype=np.float32)
    logits = xt @ gate_w.astype(np.float32).T  # [T, E]
    scores = 1.0 / (1.0 + np.exp(-logits))
    biased = scores + np.asarray(expert_bias, np.float32)[None, :]
    sel = np.argsort(-biased, axis=1, kind="stable")[:, :TOP_K]  # [T, K]
    top_scores = np.take_along_axis(scores, sel, axis=1) * ROUTE_SCALE
    sel_flat = sel.reshape(-1)
    order = np.argsort(sel_flat, kind="stable")  # [T*K]
    counts = np.bincount(sel_flat, minlength=NCORES)
    tok_idx = order // TOP_K
    scores_sorted = top_scores.reshape(-1)[order].astype(np.float32)
    return xt, counts, tok_idx, scores_sorted


def kernel(x, gate_w, w1, w2, w3, sw1, sw2, sw3, expert_bias):
    from concourse.bass_utils import run_bass_kernel_spmd

    x = np.asarray(x, np.float32)
    gate_w = np.asarray(gate_w, np.float32)
    w1 = np.asarray(w1, np.float32)
    w2 = np.asarray(w2, np.float32)
    w3 = np.asarray(w3, np.float32)
    sw1 = np.asarray(sw1, np.float32)
    sw2 = np.asarray(sw2, np.float32)
    sw3 = np.asarray(sw3, np.float32)
    expert_bias = np.asarray(expert_bias, np.float32)
    BS, SLEN, D = x.shape
    T = BS * SLEN
    H = w1.shape[1]
    TS = T // NCORES

    xt, counts, tok_idx, scores_sorted = _route(x, gate_w, expert_bias)
    off = np.concatenate([[0], np.cumsum(counts)]).astype(np.int64)
    CAP = max(128, int(math.ceil(counts.max() / 128) * 128))
    NTOK = _pick_ntok(max(128, int(counts.max())), CAP)

    key = (D, H, CAP, NTOK, TS)
    if key not in _PROGRAM_CACHE:
        _PROGRAM_CACHE[key] = _build_program(D, H, CAP, NTOK, TS)
    nc = _PROGRAM_CACHE[key]

    # stage per-core inputs
    sw1t_h = np.ascontiguousarray(np.asarray(sw1, np.float32).T).astype(BF16)
    sw3t_h = np.ascontiguousarray(np.asarray(sw3, np.float32).T).astype(BF16)
    sw2t_h = np.ascontiguousarray(np.asarray(sw2, np.float32).T).astype(BF16)
    in_maps = []
    for e in range(NCORES):
        n_e = int(counts[e])
        idx = tok_idx[off[e] : off[e] + n_e]
        seg = xt[idx] * scores_sorted[off[e] : off[e] + n_e, None]  # [n_e, D] f32
        xrT = np.zeros((D, NTOK), BF16)
        xrT[:, :n_e] = seg.T.astype(BF16)
        in_maps.append(
            {
                "xr": xrT,
                "w1t": np.ascontiguousarray(np.asarray(w1[e], np.float32).T).astype(BF16),
                "w3t": np.ascontiguousarray(np.asarray(w3[e], np.float32).T).astype(BF16),
                "w2t": np.ascontiguousarray(np.asarray(w2[e], np.float32).T).astype(BF16),
                "xs": np.ascontiguousarray(xt[e * TS : (e + 1) * TS].T).astype(BF16),
                "sw1t": sw1t_h,
                "sw3t": sw3t_h,
                "sw2t": sw2t_h,
            }
        )

    trace = os.environ.get("KERNEL_TRACE", "") not in ("", "0")
    if trace:
        _install_profhook()
    res = run_bass_kernel_spmd(
        nc, in_maps, list(range(NCORES)), trace=trace
    )
    LAST["exec_time_ns"] = res.exec_time_ns
    LAST["results"] = res

    # combine: shared slices + per-expert scatter-add
    out = np.empty((T, D), np.float32)
    for c in range(NCORES):
        out[c * TS : (c + 1) * TS] = res.results[c]["outs"]
    for e in range(NCORES):
        n_e = int(counts[e])
        if n_e:
            idx = tok_idx[off[e] : off[e] + n_e]
            out[idx] += res.results[e]["outr"][:n_e]
    return out.reshape(BS, SLEN, D)


# revision 44
# speedup vs baseline: 1.2768x; 1.0168x over previous
"""MoE (token-choice top-2 router + grouped SwiGLU experts + shared expert)
on 8 Trainium2 NeuronCores.

Sharding: expert-parallel — core e owns expert e's routed tokens (host
dispatch, capacity-padded), plus a 1/8 data-parallel slice of the shared
expert. Host does the (cheap) routing control plane: gate matmul, top-2
selection, stable sort by expert, gather/scale, and the final scatter-add
combine. The device kernel does all the FLOPs: per-core SwiGLU
  h = silu(x @ w1.T) * (x @ w3.T);  out = h @ w2.T
in bf16 with fp32 PSUM accumulation (matching the reference's bf16
grouped-mm semantics), for both the routed tokens and the shared slice.

Self-contained: only needs numpy/ml_dtypes/concourse (the Bass stack).
"""

import math
import os

import numpy as np
import ml_dtypes

BF16 = ml_dtypes.bfloat16
NCORES = 8
TOP_K = 2
ROUTE_SCALE = 1.0

# filled by the last kernel() call (exec_time_ns etc. when tracing)
LAST = {}

_PROGRAM_CACHE = {}


def _install_profhook():
    """Best-effort shim for antenv.axon_hooks so trace=True can capture NTFF
    profiles in this container. Harmless no-op if anything is missing."""
    try:
        import sys
        import types

        if "antenv.axon_hooks" in sys.modules:
            return
        import trn_agent_boot.trn_boot as tb

        hook = tb._ntff_profile_via_ctypes("/opt/axon/libaxon_pjrt.so")
        m = types.ModuleType("antenv.axon_hooks")
        m._hook = hook
        m.set_axon_ntff_profile_hook = lambda h: setattr(m, "_hook", h)
        m.get_axon_ntff_profile_hook = lambda: m._hook
        import antenv

        sys.modules["antenv.axon_hooks"] = m
        antenv.axon_hooks = m

        import concourse.bass_utils as bu

        bu.upload_artifacts = lambda tmpdir: tmpdir
    except Exception:
        pass


def _free_div(n):
    """Largest f = n/k (k<=4) with f <= 512, preferring big f."""
    for k in (1, 2, 3, 4):
        if n % k == 0 and n // k <= 512:
            return n // k
    for f in (512, 384, 256, 128):
        if n % f == 0:
            return f
    raise ValueError(f"no free-dim divisor for {n}")


def _pick_ntok(nmax, cap):
    """Smallest n in [nmax, cap] whose free-dim divides nicely (PSUM <=512)."""
    for n in range(nmax, cap + 1):
        try:
            _free_div(n)
            return n
        except ValueError:
            continue
    return cap


def _build_program(D, H, CAP, NTOK, TS):
    import concourse.bacc as bacc
    import concourse.bass as bass
    import concourse.tile as tile
    from concourse import mybir
    from concourse.kernels.tile_matmul import (
        ShapeInfo,
        batched_producer_kxm,
        composable_matmul_tile_kernel,
        dma_from_dram_kxm,
        dma_from_dram_kxn,
        dma_to_dram_mxn,
    )
    from contextlib import ExitStack

    bf = mybir.dt.bfloat16
    f32 = mybir.dt.float32
    P = 128

    nc = bacc.Bacc(target_bir_lowering=False)

    xr = nc.dram_tensor("xr", [D, NTOK], bf, kind="ExternalInput")
    w1t = nc.dram_tensor("w1t", [D, H], bf, kind="ExternalInput")
    w3t = nc.dram_tensor("w3t", [D, H], bf, kind="ExternalInput")
    w2t = nc.dram_tensor("w2t", [H, D], bf, kind="ExternalInput")
    xs = nc.dram_tensor("xs", [D, TS], bf, kind="ExternalInput")
    sw1t = nc.dram_tensor("sw1t", [D, H], bf, kind="ExternalInput")
    sw3t = nc.dram_tensor("sw3t", [D, H], bf, kind="ExternalInput")
    sw2t = nc.dram_tensor("sw2t", [H, D], bf, kind="ExternalInput")
    outr = nc.dram_tensor("outr", [CAP, D], bf, kind="ExternalOutput")
    outs = nc.dram_tensor("outs", [TS, D], f32, kind="ExternalOutput")

    with tile.TileContext(nc) as tc, ExitStack() as ctx:
        caches = ctx.enter_context(tc.tile_pool(name="caches", bufs=1))
        xcache = caches.tile([P, D // P, NTOK], bf, tag="xcache")
        xscache = caches.tile([P, D // P, TS], bf, tag="xscache")
        h1cache = caches.tile([P, H // P, NTOK], bf, tag="h1cache")
        gcache = caches.tile([P, H // P, CAP], bf, tag="gcache")
        h1scache = caches.tile([P, H // P, TS], bf, tag="h1scache")
        gscache = caches.tile([P, H // P, TS], bf, tag="gscache")

        def swiglu_h(label, w1ap, w3ap, xap, xc, h1c, gc, M_COLS):
            """h1c = silu(w1 @ x); gc = h1c * (w3 @ x). All [H, M_COLS]."""
            FREE = _free_div(M_COLS)
            kxm_pool = ctx.enter_context(tc.tile_pool(name=f"wp_{label}", bufs=7))
            p1, s1 = dma_from_dram_kxm(kxm_pool, w1ap[:])
            p3, s3 = dma_from_dram_kxm(kxm_pool, w3ap[:])
            kxm_producer, kxm_shape = batched_producer_kxm(
                [p1, p3], [s1, s3], batch_dim="m"
            )

            x3 = xap[:].rearrange("(po pi) f -> pi po f", pi=P)

            def kxn_producer(nc_, md):
                # fill the SBUF cache with one DMA per k-subtile so the first
                # matmul only waits for its own 128-row slice
                cols = bass.ts(md.n_tile_idx, md.n_tile)
                for s in range(md.k_subtiles):
                    po = md.k_tile_idx * md.k_subtiles + s
                    nc_.sync.dma_start(
                        out=xc[:, po : po + 1, cols], in_=x3[:, po : po + 1, cols]
                    )
                return xc[:, bass.ts(md.k_tile_idx, md.k_subtiles), cols]

            kxn_shape = ShapeInfo(pdims=((P, D // P),), fdims=(M_COLS,))

            def producer(nc_, md):
                c = h1c if md.m_batch_idx == 0 else gc
                return c[
                    :,
                    bass.ts(md.m_tile_idx, md.m_subtiles),
                    bass.ts(md.n_tile_idx, md.n_tile),
                ]

            def reducer(nc_, psum, sbuf, md):
                if md.m_batch_idx == 0:
                    nc_.scalar.activation(
                        sbuf, psum, mybir.ActivationFunctionType.Silu
                    )
                else:
                    start = md.n_tile_idx * md.n_tile + md.n_subtile_idx * md.n_subtile
                    sz = md.n_subtile_slice_size
                    po = md.m_tile_idx * md.m_subtiles + md.m_subtile_idx
                    nc_.vector.tensor_mul(
                        out=sbuf,
                        in0=psum[:, :sz],
                        in1=h1c[:, po, start : start + sz],
                    )

            composable_matmul_tile_kernel(
                tc=tc,
                kxm_shape=kxm_shape,
                kxn_shape=kxn_shape,
                output_type=bf,
                kxm_producer=kxm_producer,
                kxn_producer=kxn_producer,
                mxn_subtile_producer=producer,
                mxn_subtile_reducer=reducer,
                mxn_consumer=lambda nc_, sbuf, md: None,
                MATMUL_FREE_DIM=FREE,
                MAX_TILE_SIZE=max(M_COLS, 128),
                MAX_K_TILE_SIZE=512,
                psum_n_bufs=2,
            )

        def out_proj(label, gc, w2ap, out_ap, out_dt, M_COLS, max_m, big_kxn, psum_bufs=2):
            """out = (g.T @ w2.T) i.e. [M_COLS, D] = gT[H, M]^T @ w2T[H, D].
            gT lives in SBUF (gc) — kxm producer is a pure slice, no DMA."""

            def pm(nc_, md):
                return gc[
                    :,
                    bass.ts(md.k_tile_idx, md.k_subtiles),
                    bass.ts(md.m_tile_idx, md.m_tile),
                ]

            sm = ShapeInfo(pdims=((P, H // P),), fdims=(M_COLS,))

            # W2 strips in k-groups of <=4, issued from GpSimd (SP's DMA
            # descriptor-issue rate saturates in this phase otherwise)
            KT = H // P  # k-tiles (K_TILE=128)
            GRP = 4
            w2pool = ctx.enter_context(
                tc.tile_pool(name=f"w2p_{label}", bufs=2)
            )
            w2_3d = w2ap[:].rearrange("(po pi) f -> pi po f", pi=P)
            state = {"n": None, "grp": {}}

            def pn(nc_, md):
                if state["n"] != md.n_tile_idx:
                    state["n"] = md.n_tile_idx
                    state["grp"] = {}
                    cols = bass.ts(md.n_tile_idx, md.n_tile)
                    for g0 in range(0, KT, GRP):
                        g1 = min(g0 + GRP, KT)
                        t = w2pool.tile(
                            [P, g1 - g0, md.n_tile], bf, tag=f"w2g_{label}_{g0}"
                        )
                        if label == "r":
                            nc_.gpsimd.dma_start(out=t[:], in_=w2_3d[:, g0:g1, cols])
                        else:
                            nc_.scalar.dma_start(out=t[:], in_=w2_3d[:, g0:g1, cols])
                        for k in range(g0, g1):
                            state["grp"][k] = t[:, k - g0 : k - g0 + 1, :]
                return state["grp"][md.k_tile_idx]

            sn = ShapeInfo(pdims=((P, H // P),), fdims=(D,))

            out3 = out_ap[:].rearrange("(po pi) f -> pi po f", pi=P)

            def consumer(nc_, mxn_tile, md):
                n_sz = min(md.n_tile, D - md.n_tile_idx * md.n_tile)
                eng = nc_.scalar if label == "r" else nc_.sync
                eng.dma_start(
                    out=out3[
                        :,
                        bass.ts(md.m_tile_idx, md.m_subtiles),
                        bass.ds(md.n_tile_idx * md.n_tile, n_sz),
                    ],
                    in_=mxn_tile[:, :, :n_sz],
                )
            composable_matmul_tile_kernel(
                tc=tc,
                kxm_shape=sm,
                kxn_shape=sn,
                output_type=out_dt,
                kxm_producer=pm,
                kxn_producer=pn,
                mxn_consumer=consumer,
                MATMUL_FREE_DIM=512,
                MAX_TILE_SIZE=max_m,
                MAX_K_TILE_SIZE=512,
                temps_n_bufs=2,
                psum_n_bufs=psum_bufs,
            )

        swiglu_h("r", w1t, w3t, xr, xcache, h1cache, gcache, NTOK)
        swiglu_h("s", sw1t, sw3t, xs, xscache, h1scache, gscache, TS)
        out_proj("r", gcache, w2t, outr, bf, CAP, 512, big_kxn=True, psum_bufs=2)
        out_proj("s", gscache, sw2t, outs, f32, TS, 256, big_kxn=True, psum_bufs=2)

    nc.compile()
    return nc


def _route(x, gate_w, expert_bias):
    """Host control plane mirroring the reference routing exactly."""
    BS, SLEN, D = x.shape
    T = BS * SLEN
    xt = np.ascontiguousarray(x.reshape(T, D), dtype=np.float32)
    logits = xt @ gate_w.astype(np.float32).T  # [T, E]
    scores = 1.0 / (1.0 + np.exp(-logits))
    biased = scores + np.asarray(expert_bias, np.float32)[None, :]
    sel = np.argsort(-biased, axis=1, kind="stable")[:, :TOP_K]  # [T, K]
    top_scores = np.take_along_axis(scores, sel, axis=1) * ROUTE_SCALE
    sel_flat = sel.reshape(-1)
    order = np.argsort(sel_flat, kind="stable")  # [T*K]
    counts = np.bincount(sel_flat, minlength=NCORES)
    tok_idx = order // TOP_K
    scores_sorted = top_scores.reshape(-1)[order].astype(np.float32)
    return xt, counts, tok_idx, scores_sorted


def kernel(x, gate_w, w1, w2, w3, sw1, sw2, sw3, expert_bias):
    from concourse.bass_utils import run_bass_kernel_spmd

    x = np.asarray(x, np.float32)
    gate_w = np.asarray(gate_w, np.float32)
    w1 = np.asarray(w1, np.float32)
    w2 = np.asarray(w2, np.float32)
    w3 = np.asarray(w3, np.float32)
    sw1 = np.asarray(sw1, np.float32)
    sw2 = np.asarray(sw2, np.float32)
    sw3 = np.asarray(sw3, np.float32)
    expert_bias = np.asarray(expert_bias, np.float32)
    BS, SLEN, D = x.shape
    T = BS * SLEN
    H = w1.shape[1]
    TS = T // NCORES

    xt, counts, tok_idx, scores_sorted = _route(x, gate_w, expert_bias)
    off = np.concatenate([[0], np.cumsum(counts)]).astype(np.int64)
    CAP = max(128, int(math.ceil(counts.max() / 128) * 128))
    NTOK = _pick_ntok(max(128, int(counts.max())), CAP)

    key = (D, H, CAP, NTOK, TS)
    if key not in _PROGRAM_CACHE:
        _PROGRAM_CACHE[key] = _build_program(D, H, CAP, NTOK, TS)
    nc = _PROGRAM_CACHE[key]

    # stage per-core inputs
    sw1t_h = np.ascontiguousarray(np.asarray(sw1, np.float32).T).astype(BF16)
    sw3t_h = np.ascontiguousarray(np.asarray(sw3, np.float32).T).astype(BF16)
    sw2t_h = np.ascontiguousarray(np.asarray(sw2, np.float32).T).astype(BF16)
    in_maps = []
    for e in range(NCORES):
        n_e = int(counts[e])
        idx = tok_idx[off[e] : off[e] + n_e]
        seg = xt[idx] * scores_sorted[off[e] : off[e] + n_e, None]  # [n_e, D] f32
        xrT = np.zeros((D, NTOK), BF16)
        xrT[:, :n_e] = seg.T.astype(BF16)
        in_maps.append(
            {
                "xr": xrT,
                "w1t": np.ascontiguousarray(np.asarray(w1[e], np.float32).T).astype(BF16),
                "w3t": np.ascontiguousarray(np.asarray(w3[e], np.float32).T).astype(BF16),
                "w2t": np.ascontiguousarray(np.asarray(w2[e], np.float32).T).astype(BF16),
                "xs": np.ascontiguousarray(xt[e * TS : (e + 1) * TS].T).astype(BF16),
                "sw1t": sw1t_h,
                "sw3t": sw3t_h,
                "sw2t": sw2t_h,
            }
        )

    trace = os.environ.get("KERNEL_TRACE", "") not in ("", "0")
    if trace:
        _install_profhook()
    res = run_bass_kernel_spmd(
        nc, in_maps, list(range(NCORES)), trace=trace
    )
    LAST["exec_time_ns"] = res.exec_time_ns
    LAST["results"] = res

    # combine: shared slices + per-expert scatter-add
    out = np.empty((T, D), np.float32)
    for c in range(NCORES):
        out[c * TS : (c + 1) * TS] = res.results[c]["outs"]
    for e in range(NCORES):
        n_e = int(counts[e])
        if n_e:
            idx = tok_idx[off[e] : off[e] + n_e]
            out[idx] += res.results[e]["outr"][:n_e]
    return out.reshape(BS, SLEN, D)
